# revision 23
# baseline (speedup 1.0000x reference)
"""NeuralGraphPool kernel for Trainium2 (8 NeuronCores, data-parallel over batch).

Computation (per molecule b):
    out[a, f] = max(atoms[a, f], max_{d: edges[a,d]>=0} atoms[edges[a,d], f])
                * (any edge valid ? 1 : 0)

Strategy (stage 4 — mixed TensorE / DMA gather, verifier-legal ops only):
  - Shard batch B=256 across 8 cores (32 molecules each).
  - Neighbour slots 0-3 are gathered on the PE: one matmul per slot with a
    host-built one-hot matrix P^T[j, i] = (edge_s[i] == j) in fp8 against the
    fp16 atom tile (exact gather). They land in ONE 4-bank PSUM tile, exited
    by a single ScalarE Copy to fp16. Slots 4-7 come via dma_gather from an
    fp16 row table in DRAM (masked atoms read a zero row).
  - TRN2 constraints honoured: no op reads two PSUM operands; the Pool engine
    runs no ALU ops (it only issues the SWDGE gathers).
  - DVE runs the fp16 max tree two molecules per op; ScalarE applies the
    degree mask (per-partition scale) during the fp16->f32 output cast.
"""

import numpy as np

import concourse.bacc as bacc
import concourse.mybir as mybir
from concourse.tile import TileContext
from concourse.bass_utils import run_bass_kernel_spmd

# Problem constants (hardcoded per harness contract).
B, A, D, F = 256, 128, 8, 512
N_CORES = 8
BPC = B // N_CORES           # molecules per core (32)
NPE = 4                      # neighbour slots gathered on the PE
NG = 4                       # neighbour slots gathered via dma_gather
GRP = 4                      # molecules per dma_gather call
NI = GRP * NG * A            # gather indices per call (2048)
ROWS = BPC * A + 16          # atrows rows (incl. zero row at BPC*A)

_cached = {}


def _build_kernel():
    if "nc" in _cached:
        return _cached["nc"]
    nc = bacc.Bacc("TRN2", num_devices=N_CORES)
    f8 = mybir.dt.float8e4
    f16 = mybir.dt.float16
    f32 = mybir.dt.float32
    MAX = mybir.AluOpType.max

    at16 = nc.declare_dram_parameter("at16", [A, BPC * F], f16, isOutput=False)
    atrows = nc.declare_dram_parameter("atrows", [ROWS, F], f16, isOutput=False)
    pt8 = nc.declare_dram_parameter("pt8", [A, BPC * NPE * A], f8, isOutput=False)
    gidx = nc.declare_dram_parameter("gidx", [128, (BPC // GRP) * (NI // 16)],
                                     mybir.dt.int16, isOutput=False)
    maskt = nc.declare_dram_parameter("maskt", [A, BPC], f32, isOutput=False)
    out = nc.declare_dram_parameter("out", [BPC * A, F], f32, isOutput=True)

    with TileContext(nc) as tc:
        with (
            tc.tile_pool(name="const", bufs=1) as cpool,
            tc.tile_pool(name="psum", bufs=1, space="PSUM") as ppool,
            tc.tile_pool(name="g16", bufs=2) as gpool,
            tc.tile_pool(name="q16", bufs=3) as qpool,
            tc.tile_pool(name="outp", bufs=4) as opool,
        ):
            P = cpool.tile([A, BPC, NPE, A], f8)
            At = cpool.tile([A, BPC, F], f16)
            M = cpool.tile([A, BPC], f32)
            idx = cpool.tile([128, (BPC // GRP) * (NI // 16)], mybir.dt.int16)
            nc.sync.dma_start(out=M[:], in_=maskt[:])
            nc.sync.dma_start(out=idx[:], in_=gidx[:])
            pt8v = pt8[:].rearrange("j (m s i) -> j m s i", m=BPC, s=NPE)
            at16v = at16[:].rearrange("i (m f) -> i m f", m=BPC)
            # chunked uploads, small first so molecule-0 compute starts early
            bounds = [0, 1, 2, 4, 8, 16, 32]
            for lo, hi in zip(bounds[:-1], bounds[1:]):
                sl = slice(lo, hi)
                nc.sync.dma_start(out=P[:, sl], in_=pt8v[:, sl])
                nc.sync.dma_start(out=At[:, sl], in_=at16v[:, sl])

            IC = NI // 16
            G = None
            o = None
            for mp in range(BPC // 2):
                ms = (2 * mp, 2 * mp + 1)
                if ms[0] % GRP == 0:
                    grp = ms[0] // GRP
                    G = gpool.tile([A, GRP * NG, F], f16)
                    nc.gpsimd.dma_gather(
                        out_ap=G[:],
                        in_ap=atrows[:],
                        idxs_ap=idx[:, grp * IC:(grp + 1) * IC],
                        num_idxs=NI,
                        num_idxs_reg=NI,
                        elem_size=F,
                        single_packet=False,
                    )
                # PE gathers slots 0..3 of each molecule into a 4-bank tile;
                # ScalarE exits each to fp16 in one op
                q = qpool.tile([A, 2, 4, F], f16)
                for j, m in enumerate(ms):
                    AA = ppool.tile([A, 4, F], f32, name=f"aa{j}", bufs=1)
                    for s in range(NPE):
                        nc.tensor.matmul(AA[:, s, :], P[:, m, s, :], At[:, m, :],
                                         start=True, stop=True)
                    nc.scalar.activation(out=q[:, j, :, :], in_=AA[:],
                                         func=mybir.ActivationFunctionType.Copy,
                                         bias=0.0, scale=1.0)
                # DVE fp16 max tree, two molecules per op
                r = qpool.tile([A, 2, 2, F], f16)
                nc.vector.tensor_tensor(out=r[:], in0=q[:, :, 0:2, :],
                                        in1=q[:, :, 2:4, :], op=MAX)
                gv = G[:].rearrange("p (m s) f -> p m s f", s=NG)
                w = qpool.tile([A, 2, 2, F], f16)
                nc.vector.tensor_tensor(out=w[:],
                                        in0=gv[:, ms[0] % GRP:ms[0] % GRP + 2, 0:2, :],
                                        in1=gv[:, ms[0] % GRP:ms[0] % GRP + 2, 2:4, :],
                                        op=MAX)
                u = qpool.tile([A, 2, 2, F], f16)
                nc.vector.tensor_tensor(out=u[:], in0=r[:], in1=w[:], op=MAX)
                t1 = qpool.tile([A, 2, F], f16)
                nc.vector.tensor_tensor(out=t1[:], in0=u[:, :, 0, :],
                                        in1=u[:, :, 1, :], op=MAX)
                t2 = qpool.tile([A, 2, F], f16)
                nc.vector.tensor_tensor(out=t2[:], in0=t1[:],
                                        in1=At[:, ms[0]:ms[0] + 2, :], op=MAX)
                # final cast on ScalarE applies the degree mask (per-partition
                # scale); masked atoms' other inputs are zero by construction
                o = opool.tile([A, 2, F], f32)
                for j, m in enumerate(ms):
                    nc.scalar.activation(out=o[:, j, :], in_=t2[:, j, :],
                                         func=mybir.ActivationFunctionType.Copy,
                                         bias=0.0, scale=M[:, m:m + 1])
                dst = out[ms[0] * A:(ms[0] + 2) * A, :].rearrange(
                    "(m p) f -> p m f", p=A)
                nc.sync.dma_start(out=dst, in_=o[:])
    nc.compile()
    _cached["nc"] = nc
    return nc


def _host_prep(atoms, bonds, edges):
    """Build per-core input maps. atoms (B,A,F) f32; edges (B,A,D) int32."""
    del bonds  # unused by the layer
    f8np = mybir.dt.np(mybir.dt.float8e4)
    a_idx = np.arange(A, dtype=np.int64)[None, :, None]            # (1,A,1)
    e = edges.astype(np.int64)
    valid = e >= 0
    e_fixed = np.where(valid, e, a_idx)                            # (B,A,D)
    mask = valid.any(axis=2)                                       # (B,A)

    at16_full = atoms.astype(np.float16)

    in_maps = []
    jj = np.arange(A, dtype=np.int64)[:, None, None, None]         # (A,1,1,1)
    for c in range(N_CORES):
        mol = slice(c * BPC, (c + 1) * BPC)
        # pt8[j, m, s, i] = (e_fixed[m, i, s] == j), masked atoms zeroed
        ef = e_fixed[mol].transpose(0, 2, 1)                       # [m, s, i]
        ef_pe = np.where(mask[mol][:, None, :], ef[:, :NPE, :], 999)
        p8 = (ef_pe[None, :, :, :] == jj).astype(f8np)             # (A,BPC,NPE,A)
        p8 = np.ascontiguousarray(p8).reshape(A, BPC * NPE * A)
        # gather rows for slots NPE..NPE+NG-1; masked atoms -> zero row
        base = (np.arange(BPC, dtype=np.int64) * A)[:, None, None]
        rows = ef[:, NPE:NPE + NG, :] + base                       # (BPC, NG, A)
        rows = np.where(mask[mol][:, None, :], rows, BPC * A)
        rows = rows.reshape(BPC // GRP, NI).astype(np.int16)       # i=(k*NG+t)*A+a
        idx16 = rows.reshape(BPC // GRP, NI // 16, 16).transpose(0, 2, 1)
        idx16 = np.tile(idx16, (1, 8, 1)).transpose(1, 0, 2).reshape(
            128, (BPC // GRP) * (NI // 16))
        idx16 = np.ascontiguousarray(idx16)
        # fp16 atom row table (+ zero row), and the partition-major tile
        ar = np.zeros((ROWS, F), np.float16)
        ar[:BPC * A] = at16_full[mol].reshape(BPC * A, F)
        a16 = np.ascontiguousarray(
            at16_full[mol].transpose(1, 0, 2)).reshape(A, BPC * F)
        mk = np.ascontiguousarray(mask[mol].T.astype(np.float32))  # (A, BPC)
        in_maps.append({"pt8": p8, "gidx": idx16, "atrows": ar,
                        "maskt": mk, "at16": a16})
    return in_maps


def kernel(atoms, bonds, edges, _want_timing=False, **_ignored):
    nc = _build_kernel()
    in_maps = _host_prep(np.asarray(atoms, dtype=np.float32), bonds,
                         np.asarray(edges, dtype=np.int32))
    res = run_bass_kernel_spmd(nc, in_maps, list(range(N_CORES)),
                               trace=False)
    outs = [res.results[c]["out"].reshape(BPC, A, F) for c in range(N_CORES)]
    full = np.concatenate(outs, axis=0)
    if _want_timing:
        return full, res
    return full


# revision 26
# speedup vs baseline: 1.0056x; 1.0056x over previous
"""NeuralGraphPool kernel for Trainium2 (8 NeuronCores, data-parallel over batch).

Computation (per molecule b):
    out[a, f] = max(atoms[a, f], max_{d: edges[a,d]>=0} atoms[edges[a,d], f])
                * (any edge valid ? 1 : 0)

Strategy (stage 4 — mixed TensorE / DMA gather, verifier-legal ops only):
  - Shard batch B=256 across 8 cores (32 molecules each).
  - Neighbour slots 0-3 are gathered on the PE: one matmul per slot with a
    host-built one-hot matrix P^T[j, i] = (edge_s[i] == j) in fp8 against the
    fp16 atom tile (exact gather). They land in ONE 4-bank PSUM tile, exited
    by a single ScalarE Copy to fp16. Slots 4-7 come via dma_gather from an
    fp16 row table in DRAM (masked atoms read a zero row).
  - TRN2 constraints honoured: no op reads two PSUM operands; the Pool engine
    runs no ALU ops (it only issues the SWDGE gathers).
  - DVE runs the fp16 max tree two molecules per op; ScalarE applies the
    degree mask (per-partition scale) during the fp16->f32 output cast.
"""

import numpy as np

import concourse.bacc as bacc
import concourse.mybir as mybir
from concourse.tile import TileContext
from concourse.bass_utils import run_bass_kernel_spmd

# Problem constants (hardcoded per harness contract).
B, A, D, F = 256, 128, 8, 512
N_CORES = 8
BPC = B // N_CORES           # molecules per core (32)
NPE = 4                      # neighbour slots gathered on the PE
NG = 4                       # neighbour slots gathered via dma_gather
GRP = 4                      # molecules per dma_gather call
NI = GRP * NG * A            # gather indices per call (2048)
ROWS = BPC * A + 16          # atrows rows (incl. zero row at BPC*A)

_cached = {}


def _build_kernel():
    if "nc" in _cached:
        return _cached["nc"]
    nc = bacc.Bacc("TRN2", num_devices=N_CORES)
    f8 = mybir.dt.float8e4
    f16 = mybir.dt.float16
    f32 = mybir.dt.float32
    MAX = mybir.AluOpType.max
    MUL = mybir.AluOpType.mult

    at16 = nc.declare_dram_parameter("at16", [A, BPC * F], f16, isOutput=False)
    atrows = nc.declare_dram_parameter("atrows", [ROWS, F], f16, isOutput=False)
    pt8 = nc.declare_dram_parameter("pt8", [A, BPC * NPE * A], f8, isOutput=False)
    gidx = nc.declare_dram_parameter("gidx", [128, (BPC // GRP) * (NI // 16)],
                                     mybir.dt.int16, isOutput=False)
    maskt = nc.declare_dram_parameter("maskt", [A, BPC], f32, isOutput=False)
    out = nc.declare_dram_parameter("out", [BPC * A, F], f32, isOutput=True)

    with TileContext(nc) as tc:
        with (
            tc.tile_pool(name="const", bufs=1) as cpool,
            tc.tile_pool(name="psum", bufs=1, space="PSUM") as ppool,
            tc.tile_pool(name="g16", bufs=2) as gpool,
            tc.tile_pool(name="q16", bufs=4) as qpool,
            tc.tile_pool(name="outp", bufs=4) as opool,
        ):
            P = cpool.tile([A, BPC, NPE, A], f8)
            At = cpool.tile([A, BPC, F], f16)
            M = cpool.tile([A, BPC], f32)
            idx = cpool.tile([128, (BPC // GRP) * (NI // 16)], mybir.dt.int16)
            nc.sync.dma_start(out=M[:], in_=maskt[:])
            nc.sync.dma_start(out=idx[:], in_=gidx[:])
            pt8v = pt8[:].rearrange("j (m s i) -> j m s i", m=BPC, s=NPE)
            at16v = at16[:].rearrange("i (m f) -> i m f", m=BPC)
            IC = NI // 16
            NGROUPS = BPC // GRP

            def upload(lo, hi):
                sl = slice(lo, hi)
                nc.sync.dma_start(out=P[:, sl], in_=pt8v[:, sl])
                nc.sync.dma_start(out=At[:, sl], in_=at16v[:, sl])

            def gather(grp):
                Gt = gpool.tile([A, GRP * NG, F], f16)
                nc.gpsimd.dma_gather(
                    out_ap=Gt[:],
                    in_ap=atrows[:],
                    idxs_ap=idx[:, grp * IC:(grp + 1) * IC],
                    num_idxs=NI,
                    num_idxs_reg=NI,
                    elem_size=F,
                    single_packet=False,
                )
                return Gt

            # group-0 uploads in small chunks so molecule-0 compute starts
            # early; later groups are prefetched just-in-time inside the loop
            # to keep the DMA device evenly loaded across the whole span.
            for lo, hi in ((0, 1), (1, 2), (2, 4)):
                upload(lo, hi)
            g_tiles = {0: gather(0)}
            upload(GRP, 2 * GRP)

            G = None
            o = None
            for mp in range(BPC // 2):
                ms = (2 * mp, 2 * mp + 1)
                if ms[0] % GRP == 0:
                    grp = ms[0] // GRP
                    if grp + 1 < NGROUPS:
                        g_tiles[grp + 1] = gather(grp + 1)
                    if grp + 2 < NGROUPS:
                        upload((grp + 2) * GRP, (grp + 3) * GRP)
                    G = g_tiles.pop(grp)
                # PE gathers slots 0..3 of each molecule into a 4-bank tile;
                # ScalarE exits each to fp16 in one op
                q = qpool.tile([A, 2, 4, F], f16)
                for j, m in enumerate(ms):
                    AA = ppool.tile([A, 4, F], f32, name=f"aa{j}", bufs=1)
                    for s in range(NPE):
                        nc.tensor.matmul(AA[:, s, :], P[:, m, s, :], At[:, m, :],
                                         start=True, stop=True)
                    nc.scalar.activation(out=q[:, j, :, :], in_=AA[:],
                                         func=mybir.ActivationFunctionType.Copy,
                                         bias=0.0, scale=1.0)
                # DVE fp16 max tree, two molecules per op
                r = qpool.tile([A, 2, 2, F], f16)
                nc.vector.tensor_tensor(out=r[:], in0=q[:, :, 0:2, :],
                                        in1=q[:, :, 2:4, :], op=MAX)
                gv = G[:].rearrange("p (m s) f -> p m s f", s=NG)
                w = qpool.tile([A, 2, 2, F], f16)
                nc.vector.tensor_tensor(out=w[:],
                                        in0=gv[:, ms[0] % GRP:ms[0] % GRP + 2, 0:2, :],
                                        in1=gv[:, ms[0] % GRP:ms[0] % GRP + 2, 2:4, :],
                                        op=MAX)
                u = qpool.tile([A, 2, 2, F], f16)
                nc.vector.tensor_tensor(out=u[:], in0=r[:], in1=w[:], op=MAX)
                t1 = qpool.tile([A, 2, F], f16)
                nc.vector.tensor_tensor(out=t1[:], in0=u[:, :, 0, :],
                                        in1=u[:, :, 1, :], op=MAX)
                # final masked self-merge + f32 cast, split across engines to
                # balance the ACT/DVE load: molecule 0 of the pair goes via a
                # DVE scalar_tensor_tensor (max(mask*self, t1) -> f32),
                # molecule 1 via fp16 merge + masked ScalarE cast. Masked
                # atoms' other inputs are zero by construction either way.
                o = opool.tile([A, 2, F], f32)
                nc.vector.scalar_tensor_tensor(out=o[:, 0, :],
                                               in0=At[:, ms[0], :],
                                               scalar=M[:, ms[0]:ms[0] + 1],
                                               in1=t1[:, 0, :],
                                               op0=MUL, op1=MAX)
                t2 = qpool.tile([A, F], f16)
                nc.vector.tensor_tensor(out=t2[:], in0=t1[:, 1, :],
                                        in1=At[:, ms[1], :], op=MAX)
                nc.scalar.activation(out=o[:, 1, :], in_=t2[:],
                                     func=mybir.ActivationFunctionType.Copy,
                                     bias=0.0, scale=M[:, ms[1]:ms[1] + 1])
                dst = out[ms[0] * A:(ms[0] + 2) * A, :].rearrange(
                    "(m p) f -> p m f", p=A)
                nc.sync.dma_start(out=dst, in_=o[:])
    nc.compile()
    _cached["nc"] = nc
    return nc


def _host_prep(atoms, bonds, edges):
    """Build per-core input maps. atoms (B,A,F) f32; edges (B,A,D) int32."""
    del bonds  # unused by the layer
    f8np = mybir.dt.np(mybir.dt.float8e4)
    a_idx = np.arange(A, dtype=np.int64)[None, :, None]            # (1,A,1)
    e = edges.astype(np.int64)
    valid = e >= 0
    e_fixed = np.where(valid, e, a_idx)                            # (B,A,D)
    mask = valid.any(axis=2)                                       # (B,A)

    at16_full = atoms.astype(np.float16)

    in_maps = []
    jj = np.arange(A, dtype=np.int64)[:, None, None, None]         # (A,1,1,1)
    for c in range(N_CORES):
        mol = slice(c * BPC, (c + 1) * BPC)
        # pt8[j, m, s, i] = (e_fixed[m, i, s] == j), masked atoms zeroed
        ef = e_fixed[mol].transpose(0, 2, 1)                       # [m, s, i]
        ef_pe = np.where(mask[mol][:, None, :], ef[:, :NPE, :], 999)
        p8 = (ef_pe[None, :, :, :] == jj).astype(f8np)             # (A,BPC,NPE,A)
        p8 = np.ascontiguousarray(p8).reshape(A, BPC * NPE * A)
        # gather rows for slots NPE..NPE+NG-1; masked atoms -> zero row
        base = (np.arange(BPC, dtype=np.int64) * A)[:, None, None]
        rows = ef[:, NPE:NPE + NG, :] + base                       # (BPC, NG, A)
        rows = np.where(mask[mol][:, None, :], rows, BPC * A)
        rows = rows.reshape(BPC // GRP, NI).astype(np.int16)       # i=(k*NG+t)*A+a
        idx16 = rows.reshape(BPC // GRP, NI // 16, 16).transpose(0, 2, 1)
        idx16 = np.tile(idx16, (1, 8, 1)).transpose(1, 0, 2).reshape(
            128, (BPC // GRP) * (NI // 16))
        idx16 = np.ascontiguousarray(idx16)
        # fp16 atom row table (+ zero row), and the partition-major tile
        ar = np.zeros((ROWS, F), np.float16)
        ar[:BPC * A] = at16_full[mol].reshape(BPC * A, F)
        a16 = np.ascontiguousarray(
            at16_full[mol].transpose(1, 0, 2)).reshape(A, BPC * F)
        mk = np.ascontiguousarray(mask[mol].T.astype(np.float32))  # (A, BPC)
        in_maps.append({"pt8": p8, "gidx": idx16, "atrows": ar,
                        "maskt": mk, "at16": a16})
    return in_maps


def kernel(atoms, bonds, edges, _want_timing=False, **_ignored):
    nc = _build_kernel()
    in_maps = _host_prep(np.asarray(atoms, dtype=np.float32), bonds,
                         np.asarray(edges, dtype=np.int32))
    res = run_bass_kernel_spmd(nc, in_maps, list(range(N_CORES)),
                               trace=False)
    outs = [res.results[c]["out"].reshape(BPC, A, F) for c in range(N_CORES)]
    full = np.concatenate(outs, axis=0)
    if _want_timing:
        return full, res
    return full
